# revision 2
# baseline (speedup 1.0000x reference)
"""Chamfer loss kernel for Trainium2 (8 NeuronCores, batch-parallel).

Problem: target_points [16, 4096, 2], actual_points [16, 4096, 2] (fp32).
  d[b,m,n] = || t[b,m] - a[b,n] ||
  forward_loss[b,m]  = min_n d[b,m,n]
  backward_loss[b,n] = min_m d[b,m,n]

Strategy
--------
Shard batch B=16 across 8 cores (2 batches/core). On each core, compute the
squared-distance matrix with the PE via the bilinear identity
    d2[m,n] = |t_m|^2 + |a_n|^2 - 2 t_m . a_n
expressed as a K=18 matmul: each fp32 scalar is split into 3 bf16 limbs
(hi/mid/lo, 24 mantissa bits total) so bf16 matmuls (1 cyc/row on PE, vs 2
for fp32) reproduce fp32-level precision. Cross products keep the terms
{hh, hm, mh, hl, lh, mm} (dropped terms are O(2^-24)).

Both orientations (targets-stationary and actuals-stationary) are computed so
that every min-reduction is a free-axis DVE tensor_reduce over PSUM tiles.
sqrt is applied only to the final reduced [128, 32] tiles (sqrt is monotonic).
"""

import numpy as np
import ml_dtypes

B, M, N = 16, 4096, 4096
NCORES = 8
BPC = B // NCORES          # batches per core
F = BPC * M                # free width of aug arrays per core
K = 18                     # contraction rows
NB = M // 128              # m-blocks per batch (32)
HALF = 2048                # psum tile free width
BF16 = ml_dtypes.bfloat16

_CACHE = {}


def _build_nc():
    import concourse.mybir as mybir
    import concourse.tile as tile
    from concourse import bacc

    nc = bacc.Bacc(None, target_bir_lowering=False)
    taug_d = nc.declare_dram_parameter("taug", [K, F], mybir.dt.bfloat16, isOutput=False)
    aaug_d = nc.declare_dram_parameter("aaug", [K, F], mybir.dt.bfloat16, isOutput=False)
    fwd_d = nc.declare_dram_parameter("fwd", [BPC, 128, NB], mybir.dt.float32, isOutput=True)
    bwd_d = nc.declare_dram_parameter("bwd", [BPC, 128, NB], mybir.dt.float32, isOutput=True)

    fmin = mybir.AluOpType.min
    ax_x = mybir.AxisListType.X

    with tile.TileContext(nc) as tc:
        with (
            tc.tile_pool(name="aug", bufs=1) as augp,
            tc.tile_pool(name="ps", bufs=2, space="PSUM") as psp,
            tc.tile_pool(name="pm", bufs=2) as pmp,
            tc.tile_pool(name="fin", bufs=2) as finp,
        ):
            ta = augp.tile([K, F], mybir.dt.bfloat16, tag="ta")
            aa = augp.tile([K, F], mybir.dt.bfloat16, tag="aa")
            nc.sync.dma_start(out=ta[:], in_=taug_d[:])
            nc.sync.dma_start(out=aa[:], in_=aaug_d[:])

            for b in range(BPC):
                for stat, mov, outd in ((ta, aa, fwd_d), (aa, ta, bwd_d)):
                    pm = pmp.tile([128, 2 * NB], mybir.dt.float32, tag="pm")
                    for i in range(NB):
                        lhsT = stat[:, b * M + i * 128 : b * M + (i + 1) * 128]
                        for h in range(2):
                            ps = psp.tile([128, HALF], mybir.dt.float32, tag="ps")
                            for j in range(4):
                                nc.tensor.matmul(
                                    ps[:, j * 512 : (j + 1) * 512],
                                    lhsT,
                                    mov[:, b * M + h * HALF + j * 512 : b * M + h * HALF + (j + 1) * 512],
                                    start=True,
                                    stop=True,
                                )
                            nc.vector.tensor_reduce(
                                out=pm[:, 2 * i + h : 2 * i + h + 1],
                                in_=ps[:],
                                axis=ax_x,
                                op=fmin,
                            )
                    fm = finp.tile([128, NB], mybir.dt.float32, tag="fm")
                    nc.vector.tensor_reduce(
                        out=fm[:],
                        in_=pm[:].rearrange("p (i h) -> p i h", h=2),
                        axis=ax_x,
                        op=fmin,
                    )
                    fc = finp.tile([128, NB], mybir.dt.float32, tag="fc")
                    nc.vector.tensor_scalar_max(out=fc[:], in0=fm[:], scalar1=0.0)
                    fs = finp.tile([128, NB], mybir.dt.float32, tag="fs")
                    nc.scalar.activation(out=fs[:], in_=fc[:], func=mybir.ActivationFunctionType.Sqrt)
                    nc.sync.dma_start(out=outd[b], in_=fs[:])

    nc.finalize()
    return nc


def _split3(v):
    """3-way bf16 limb split of fp64 array: h + m + l == v to ~24 mantissa bits."""
    h = v.astype(BF16)
    r = v - h.astype(np.float64)
    m = r.astype(BF16)
    r2 = r - m.astype(np.float64)
    l = r2.astype(BF16)
    return h, m, l


def _make_augs(tp, ap):
    """tp, ap: [nb, M, 2] fp32 -> (taug, aaug) [K, nb*M] bf16."""
    t = tp.astype(np.float64).transpose(2, 0, 1).reshape(2, -1)  # [coord, nb*M]
    a = ap.astype(np.float64).transpose(2, 0, 1).reshape(2, -1)
    n = t.shape[1]

    txh, txm, txl = _split3(t[0])
    tyh, tym, tyl = _split3(t[1])
    t2h, t2m, t2l = _split3(t[0] ** 2 + t[1] ** 2)
    Xh, Xm, Xl = _split3(-2.0 * a[0])
    Yh, Ym, Yl = _split3(-2.0 * a[1])
    a2h, a2m, a2l = _split3(a[0] ** 2 + a[1] ** 2)
    one = np.ones(n, dtype=BF16)

    # Product pairs ordered so the PE's in-instruction fp32 accumulation sees
    # the large terms first (partial sum collapses to ~d2 after k=3, so later
    # roundings happen at small magnitude): t2_h, hh cross terms, a2_h, then
    # the mid/lo correction limbs {hm, mh, hl, lh, mm}.
    taug = np.stack([
        t2h, txh, tyh, one,
        t2m, txh, txm, tyh, tym, one,
        txh, txl, txm, tyh, tyl, tym,
        t2l, one,
    ])
    aaug = np.stack([
        one, Xh, Yh, a2h,
        one, Xm, Xh, Ym, Yh, a2m,
        Xl, Xh, Xm, Yl, Yh, Ym,
        one, a2l,
    ])
    return np.ascontiguousarray(taug), np.ascontiguousarray(aaug)


def run(target_points, actual_points, trace=False, tmpdir=None):
    from concourse.bass_utils import run_bass_kernel_spmd

    tp = np.asarray(target_points, dtype=np.float32)
    ap = np.asarray(actual_points, dtype=np.float32)
    assert tp.shape == (B, M, 2) and ap.shape == (B, N, 2)

    if "nc" not in _CACHE:
        _CACHE["nc"] = _build_nc()
    nc = _CACHE["nc"]

    in_maps = []
    for c in range(NCORES):
        taug, aaug = _make_augs(tp[BPC * c : BPC * (c + 1)], ap[BPC * c : BPC * (c + 1)])
        in_maps.append({"taug": taug, "aaug": aaug})

    res = run_bass_kernel_spmd(
        nc, in_maps, core_ids=list(range(NCORES)), trace=trace, tmpdir=tmpdir
    )

    fwd = np.empty((B, M), dtype=np.float32)
    bwd = np.empty((B, N), dtype=np.float32)
    for c in range(NCORES):
        # device layout [BPC, 128, NB]: element (b, p, i) -> index i*128 + p
        fwd[BPC * c : BPC * (c + 1)] = (
            res.results[c]["fwd"].transpose(0, 2, 1).reshape(BPC, M)
        )
        bwd[BPC * c : BPC * (c + 1)] = (
            res.results[c]["bwd"].transpose(0, 2, 1).reshape(BPC, N)
        )
    return (fwd, bwd), res


def kernel(target_points, actual_points):
    (fwd, bwd), _ = run(target_points, actual_points)
    return fwd, bwd


# revision 3
# speedup vs baseline: 1.4149x; 1.4149x over previous
"""Chamfer loss kernel for Trainium2 (8 NeuronCores, batch-parallel).

Problem: target_points [16, 4096, 2], actual_points [16, 4096, 2] (fp32).
  d[b,m,n] = || t[b,m] - a[b,n] ||
  forward_loss[b,m]  = min_n d[b,m,n]
  backward_loss[b,n] = min_m d[b,m,n]

Strategy
--------
Shard batch B=16 across 8 cores (2 batches/core). On each core, compute the
squared-distance matrix ONCE (single orientation, targets stationary) with
the PE via the bilinear identity
    d2[m,n] = |t_m|^2 + |a_n|^2 - 2 t_m . a_n
expressed as a K=18 matmul: each fp32 scalar is split into 3 bf16 limbs
(hi/mid/lo) so bf16 matmuls (1 cyc/row on PE) reproduce fp32-level
precision; limb products are ordered large-first so PSUM accumulation
rounds at small magnitude near the minima.

Per [128m x 4096n] block:
  - ScalarE evacuates PSUM to an fp16 SBUF tile (the only other engine that
    can read PSUM, freeing the DVE).
  - forward:  DVE tt-min of the two halves (2x packed) + 1x tensor_reduce.
  - backward: DVE tt-min accumulation into a per-batch [128, 4096] running
    column-min (2x packed).
Backward finalization: negate, GpSimd partition_all_reduce(max) across the
128 partitions, then sqrt(-x) on ScalarE. sqrt only touches final vectors
(sqrt is monotonic, so mins commute with it).
"""

import numpy as np
import ml_dtypes

B, M, N = 16, 4096, 4096
NCORES = 8
BPC = B // NCORES          # batches per core
F = BPC * M                # free width of aug arrays per core
K = 18                     # contraction rows
NB = M // 128              # m-blocks per batch (32)
HALF = 2048                # psum tile free width
BF16 = ml_dtypes.bfloat16

_CACHE = {}


def _build_nc():
    import concourse.mybir as mybir
    import concourse.tile as tile
    from concourse import bacc, bass_isa

    nc = bacc.Bacc(None, target_bir_lowering=False)
    taug_d = nc.declare_dram_parameter("taug", [K, F], mybir.dt.bfloat16, isOutput=False)
    aaug_d = nc.declare_dram_parameter("aaug", [K, F], mybir.dt.bfloat16, isOutput=False)
    fwd_d = nc.declare_dram_parameter("fwd", [BPC, 128, NB], mybir.dt.float32, isOutput=True)
    bwd_d = nc.declare_dram_parameter("bwd", [BPC, N], mybir.dt.float32, isOutput=True)

    f32 = mybir.dt.float32
    f16 = mybir.dt.float16
    fmin = mybir.AluOpType.min
    ax_x = mybir.AxisListType.X
    FCopy = mybir.ActivationFunctionType.Copy
    FSqrt = mybir.ActivationFunctionType.Sqrt

    with tile.TileContext(nc) as tc:
        with (
            tc.tile_pool(name="aug", bufs=1) as augp,
            tc.tile_pool(name="ps", bufs=2, space="PSUM") as psp,
            tc.tile_pool(name="e16", bufs=3) as e16p,
            tc.tile_pool(name="cmb", bufs=3) as cmbp,
            tc.tile_pool(name="accb", bufs=2) as accbp,
            tc.tile_pool(name="pm", bufs=2) as pmp,
            tc.tile_pool(name="fin", bufs=2) as finp,
        ):
            ta = augp.tile([K, F], mybir.dt.bfloat16, tag="ta")
            aa = augp.tile([K, F], mybir.dt.bfloat16, tag="aa")
            nc.sync.dma_start(out=ta[:], in_=taug_d[:])
            nc.sync.dma_start(out=aa[:], in_=aaug_d[:])

            for b in range(BPC):
                pm = pmp.tile([128, NB], f32, tag="pm")
                acc = accbp.tile([128, N], f16, tag="acc")
                nc.vector.memset(acc[:], 60000.0)
                for i in range(NB):
                    lhsT = ta[:, b * M + i * 128 : b * M + (i + 1) * 128]
                    e16 = e16p.tile([128, N], f16, tag="e16")
                    for h in range(2):
                        ps = psp.tile([128, HALF], f32, tag="ps")
                        for j in range(4):
                            nc.tensor.matmul(
                                ps[:, j * 512 : (j + 1) * 512],
                                lhsT,
                                aa[:, b * M + h * HALF + j * 512 : b * M + h * HALF + (j + 1) * 512],
                                start=True,
                                stop=True,
                            )
                        nc.scalar.activation(
                            out=e16[:, h * HALF : (h + 1) * HALF], in_=ps[:], func=FCopy
                        )
                    # forward: fold halves at 2x, then 1x reduce of the half-width
                    c = cmbp.tile([128, HALF], f16, tag="c")
                    nc.vector.tensor_tensor(
                        out=c[:], in0=e16[:, 0:HALF], in1=e16[:, HALF:N], op=fmin
                    )
                    nc.vector.tensor_reduce(
                        out=pm[:, i : i + 1], in_=c[:], axis=ax_x, op=fmin
                    )
                    # backward: running column-min
                    nc.vector.tensor_tensor(out=acc[:], in0=acc[:], in1=e16[:], op=fmin)

                # forward finalize: clamp + sqrt, out layout [128, NB] (host transposes)
                fc = finp.tile([128, NB], f32, tag="fc")
                nc.vector.tensor_scalar_max(out=fc[:], in0=pm[:], scalar1=0.0)
                fs = finp.tile([128, NB], f32, tag="fs")
                nc.scalar.activation(out=fs[:], in_=fc[:], func=FSqrt)
                nc.sync.dma_start(out=fwd_d[b], in_=fs[:])

                # backward finalize: negate, cross-partition max, sqrt(-x)
                nacc = accbp.tile([128, N], f16, tag="nacc")
                nc.vector.tensor_scalar_mul(out=nacc[:], in0=acc[:], scalar1=-1.0)
                par = accbp.tile([128, N], f16, tag="par")
                nc.gpsimd.partition_all_reduce(
                    par[:], nacc[:], channels=128, reduce_op=bass_isa.ReduceOp.max
                )
                row = finp.tile([1, N], f16, tag="row")
                nc.vector.tensor_scalar_min(out=row[:], in0=par[0:1, :], scalar1=0.0)
                brow = finp.tile([1, N], f32, tag="brow")
                nc.scalar.activation(out=brow[:], in_=row[:], func=FSqrt, scale=-1.0)
                nc.sync.dma_start(out=bwd_d[b], in_=brow[:])

    nc.finalize()
    return nc


def _split3(v):
    """3-way bf16 limb split of fp64 array: h + m + l == v to ~24 mantissa bits."""
    h = v.astype(BF16)
    r = v - h.astype(np.float64)
    m = r.astype(BF16)
    r2 = r - m.astype(np.float64)
    l = r2.astype(BF16)
    return h, m, l


def _make_augs(tp, ap):
    """tp, ap: [nb, M, 2] fp32 -> (taug, aaug) [K, nb*M] bf16."""
    t = tp.astype(np.float64).transpose(2, 0, 1).reshape(2, -1)  # [coord, nb*M]
    a = ap.astype(np.float64).transpose(2, 0, 1).reshape(2, -1)
    n = t.shape[1]

    txh, txm, txl = _split3(t[0])
    tyh, tym, tyl = _split3(t[1])
    t2h, t2m, t2l = _split3(t[0] ** 2 + t[1] ** 2)
    Xh, Xm, Xl = _split3(-2.0 * a[0])
    Yh, Ym, Yl = _split3(-2.0 * a[1])
    a2h, a2m, a2l = _split3(a[0] ** 2 + a[1] ** 2)
    one = np.ones(n, dtype=BF16)

    # Product pairs ordered so the PE's in-instruction fp32 accumulation sees
    # the large terms first (partial sum collapses to ~d2 after k=3, so later
    # roundings happen at small magnitude): t2_h, hh cross terms, a2_h, then
    # the mid/lo correction limbs {hm, mh, hl, lh, mm}.
    taug = np.stack([
        t2h, txh, tyh, one,
        t2m, txh, txm, tyh, tym, one,
        txh, txl, txm, tyh, tyl, tym,
        t2l, one,
    ])
    aaug = np.stack([
        one, Xh, Yh, a2h,
        one, Xm, Xh, Ym, Yh, a2m,
        Xl, Xh, Xm, Yl, Yh, Ym,
        one, a2l,
    ])
    return np.ascontiguousarray(taug), np.ascontiguousarray(aaug)


def run(target_points, actual_points, trace=False, tmpdir=None):
    from concourse.bass_utils import run_bass_kernel_spmd

    tp = np.asarray(target_points, dtype=np.float32)
    ap = np.asarray(actual_points, dtype=np.float32)
    assert tp.shape == (B, M, 2) and ap.shape == (B, N, 2)

    if "nc" not in _CACHE:
        _CACHE["nc"] = _build_nc()
    nc = _CACHE["nc"]

    in_maps = []
    for c in range(NCORES):
        taug, aaug = _make_augs(tp[BPC * c : BPC * (c + 1)], ap[BPC * c : BPC * (c + 1)])
        in_maps.append({"taug": taug, "aaug": aaug})

    res = run_bass_kernel_spmd(
        nc, in_maps, core_ids=list(range(NCORES)), trace=trace, tmpdir=tmpdir
    )

    fwd = np.empty((B, M), dtype=np.float32)
    bwd = np.empty((B, N), dtype=np.float32)
    for c in range(NCORES):
        # fwd device layout [BPC, 128, NB]: element (b, p, i) -> index i*128 + p
        fwd[BPC * c : BPC * (c + 1)] = (
            res.results[c]["fwd"].transpose(0, 2, 1).reshape(BPC, M)
        )
        bwd[BPC * c : BPC * (c + 1)] = res.results[c]["bwd"]
    return (fwd, bwd), res


def kernel(target_points, actual_points):
    (fwd, bwd), _ = run(target_points, actual_points)
    return fwd, bwd


# revision 4
# speedup vs baseline: 1.5709x; 1.1102x over previous
"""Chamfer loss kernel for Trainium2 (8 NeuronCores, batch-parallel).

Problem: target_points [16, 4096, 2], actual_points [16, 4096, 2] (fp32).
  d[b,m,n] = || t[b,m] - a[b,n] ||
  forward_loss[b,m]  = min_n d[b,m,n]
  backward_loss[b,n] = min_m d[b,m,n]

Strategy
--------
Shard batch B=16 across 8 cores (2 batches/core). On each core, compute the
squared-distance matrix ONCE (single orientation, targets stationary) with
the PE via the bilinear identity
    d2[m,n] = |t_m|^2 + |a_n|^2 - 2 t_m . a_n
expressed as a K=18 matmul: each fp32 scalar is split into 3 bf16 limbs
(hi/mid/lo) so bf16 matmuls (1 cyc/row on PE) reproduce fp32-level
precision; limb products are ordered large-first so PSUM accumulation
rounds at small magnitude near the minima.

Per [128m x 4096n] block:
  - ScalarE evacuates PSUM to an fp16 SBUF tile (the only other engine that
    can read PSUM, freeing the DVE).
  - forward:  DVE tt-min of the two halves (2x packed) + 1x tensor_reduce.
  - backward: DVE tt-min accumulation into a per-batch [128, 4096] running
    column-min (2x packed).
Backward finalization: negate, GpSimd partition_all_reduce(max) across the
128 partitions, then sqrt(-x) on ScalarE. sqrt only touches final vectors
(sqrt is monotonic, so mins commute with it).
"""

import numpy as np
import ml_dtypes

B, M, N = 16, 4096, 4096
NCORES = 8
BPC = B // NCORES          # batches per core
F = BPC * M                # free width of aug arrays per core
K = 18                     # contraction rows
NB = M // 128              # m-blocks per batch (32)
HALF = 2048                # psum tile free width
BF16 = ml_dtypes.bfloat16

_CACHE = {}


def _build_nc():
    import concourse.mybir as mybir
    import concourse.tile as tile
    from concourse import bacc, bass_isa

    nc = bacc.Bacc(None, target_bir_lowering=False)
    taug_d = nc.declare_dram_parameter("taug", [K, F], mybir.dt.bfloat16, isOutput=False)
    aaug_d = nc.declare_dram_parameter("aaug", [K, F], mybir.dt.bfloat16, isOutput=False)
    fwd_d = nc.declare_dram_parameter("fwd", [BPC, 128, NB], mybir.dt.float32, isOutput=True)
    bwd_d = nc.declare_dram_parameter("bwd", [BPC, N], mybir.dt.float32, isOutput=True)

    f32 = mybir.dt.float32
    f16 = mybir.dt.float16
    fmin = mybir.AluOpType.min
    ax_x = mybir.AxisListType.X
    FCopy = mybir.ActivationFunctionType.Copy
    FSqrt = mybir.ActivationFunctionType.Sqrt

    with tile.TileContext(nc) as tc:
        with (
            tc.tile_pool(name="aug", bufs=1) as augp,
            tc.tile_pool(name="ps", bufs=2, space="PSUM") as psp,
            tc.tile_pool(name="e16", bufs=3) as e16p,
            tc.tile_pool(name="cmb", bufs=3) as cmbp,
            tc.tile_pool(name="accb", bufs=2) as accbp,
            tc.tile_pool(name="pm", bufs=2) as pmp,
            tc.tile_pool(name="fin", bufs=2) as finp,
        ):
            ta = augp.tile([K, F], mybir.dt.bfloat16, tag="ta")
            aa = augp.tile([K, F], mybir.dt.bfloat16, tag="aa")
            nc.sync.dma_start(out=ta[:], in_=taug_d[:])
            nc.sync.dma_start(out=aa[:], in_=aaug_d[:])

            for b in range(BPC):
                pm = pmp.tile([128, NB], f32, tag="pm")
                acc = accbp.tile([128, N], f16, tag="acc")
                nc.vector.memset(acc[:], 60000.0)
                for i in range(NB):
                    lhsT = ta[:, b * M + i * 128 : b * M + (i + 1) * 128]
                    e16 = e16p.tile([128, N], f16, tag="e16")
                    for h in range(2):
                        ps = psp.tile([128, HALF], f32, tag="ps")
                        for j in range(4):
                            nc.tensor.matmul(
                                ps[:, j * 512 : (j + 1) * 512],
                                lhsT,
                                aa[:, b * M + h * HALF + j * 512 : b * M + h * HALF + (j + 1) * 512],
                                start=True,
                                stop=True,
                            )
                        nc.scalar.activation(
                            out=e16[:, h * HALF : (h + 1) * HALF], in_=ps[:], func=FCopy
                        )
                    # forward: fold tree at 2x-packed TT rates (4 reads/cyc),
                    # then a 1x reduce only on the final 512 columns
                    c = cmbp.tile([128, HALF], f16, tag="c")
                    nc.vector.tensor_tensor(
                        out=c[:], in0=e16[:, 0:HALF], in1=e16[:, HALF:N], op=fmin
                    )
                    c2 = cmbp.tile([128, 1024], f16, tag="c2")
                    nc.vector.tensor_tensor(
                        out=c2[:], in0=c[:, 0:1024], in1=c[:, 1024:HALF], op=fmin
                    )
                    c3 = cmbp.tile([128, 512], f16, tag="c3")
                    nc.vector.tensor_tensor(
                        out=c3[:], in0=c2[:, 0:512], in1=c2[:, 512:1024], op=fmin
                    )
                    nc.vector.tensor_reduce(
                        out=pm[:, i : i + 1], in_=c3[:], axis=ax_x, op=fmin
                    )
                    # backward: running column-min
                    nc.vector.tensor_tensor(out=acc[:], in0=acc[:], in1=e16[:], op=fmin)

                # forward finalize: clamp + sqrt, out layout [128, NB] (host transposes)
                fc = finp.tile([128, NB], f32, tag="fc")
                nc.vector.tensor_scalar_max(out=fc[:], in0=pm[:], scalar1=0.0)
                fs = finp.tile([128, NB], f32, tag="fs")
                nc.scalar.activation(out=fs[:], in_=fc[:], func=FSqrt)
                nc.sync.dma_start(out=fwd_d[b], in_=fs[:])

                # backward finalize: negate, cross-partition max, sqrt(-x)
                nacc = accbp.tile([128, N], f16, tag="nacc")
                nc.vector.tensor_scalar_mul(out=nacc[:], in0=acc[:], scalar1=-1.0)
                par = accbp.tile([128, N], f16, tag="par")
                nc.gpsimd.partition_all_reduce(
                    par[:], nacc[:], channels=128, reduce_op=bass_isa.ReduceOp.max
                )
                row = finp.tile([1, N], f16, tag="row")
                nc.vector.tensor_scalar_min(out=row[:], in0=par[0:1, :], scalar1=0.0)
                brow = finp.tile([1, N], f32, tag="brow")
                nc.scalar.activation(out=brow[:], in_=row[:], func=FSqrt, scale=-1.0)
                nc.sync.dma_start(out=bwd_d[b], in_=brow[:])

    nc.finalize()
    return nc


def _split3(v):
    """3-way bf16 limb split of fp64 array: h + m + l == v to ~24 mantissa bits."""
    h = v.astype(BF16)
    r = v - h.astype(np.float64)
    m = r.astype(BF16)
    r2 = r - m.astype(np.float64)
    l = r2.astype(BF16)
    return h, m, l


def _make_augs(tp, ap):
    """tp, ap: [nb, M, 2] fp32 -> (taug, aaug) [K, nb*M] bf16."""
    t = tp.astype(np.float64).transpose(2, 0, 1).reshape(2, -1)  # [coord, nb*M]
    a = ap.astype(np.float64).transpose(2, 0, 1).reshape(2, -1)
    n = t.shape[1]

    txh, txm, txl = _split3(t[0])
    tyh, tym, tyl = _split3(t[1])
    t2h, t2m, t2l = _split3(t[0] ** 2 + t[1] ** 2)
    Xh, Xm, Xl = _split3(-2.0 * a[0])
    Yh, Ym, Yl = _split3(-2.0 * a[1])
    a2h, a2m, a2l = _split3(a[0] ** 2 + a[1] ** 2)
    one = np.ones(n, dtype=BF16)

    # Product pairs ordered so the PE's in-instruction fp32 accumulation sees
    # the large terms first (partial sum collapses to ~d2 after k=3, so later
    # roundings happen at small magnitude): t2_h, hh cross terms, a2_h, then
    # the mid/lo correction limbs {hm, mh, hl, lh, mm}.
    taug = np.stack([
        t2h, txh, tyh, one,
        t2m, txh, txm, tyh, tym, one,
        txh, txl, txm, tyh, tyl, tym,
        t2l, one,
    ])
    aaug = np.stack([
        one, Xh, Yh, a2h,
        one, Xm, Xh, Ym, Yh, a2m,
        Xl, Xh, Xm, Yl, Yh, Ym,
        one, a2l,
    ])
    return np.ascontiguousarray(taug), np.ascontiguousarray(aaug)


def run(target_points, actual_points, trace=False, tmpdir=None):
    from concourse.bass_utils import run_bass_kernel_spmd

    tp = np.asarray(target_points, dtype=np.float32)
    ap = np.asarray(actual_points, dtype=np.float32)
    assert tp.shape == (B, M, 2) and ap.shape == (B, N, 2)

    if "nc" not in _CACHE:
        _CACHE["nc"] = _build_nc()
    nc = _CACHE["nc"]

    in_maps = []
    for c in range(NCORES):
        taug, aaug = _make_augs(tp[BPC * c : BPC * (c + 1)], ap[BPC * c : BPC * (c + 1)])
        in_maps.append({"taug": taug, "aaug": aaug})

    res = run_bass_kernel_spmd(
        nc, in_maps, core_ids=list(range(NCORES)), trace=trace, tmpdir=tmpdir
    )

    fwd = np.empty((B, M), dtype=np.float32)
    bwd = np.empty((B, N), dtype=np.float32)
    for c in range(NCORES):
        # fwd device layout [BPC, 128, NB]: element (b, p, i) -> index i*128 + p
        fwd[BPC * c : BPC * (c + 1)] = (
            res.results[c]["fwd"].transpose(0, 2, 1).reshape(BPC, M)
        )
        bwd[BPC * c : BPC * (c + 1)] = res.results[c]["bwd"]
    return (fwd, bwd), res


def kernel(target_points, actual_points):
    (fwd, bwd), _ = run(target_points, actual_points)
    return fwd, bwd


# revision 6
# speedup vs baseline: 1.5872x; 1.0104x over previous
"""Chamfer loss kernel for Trainium2 (8 NeuronCores, batch-parallel).

Problem: target_points [16, 4096, 2], actual_points [16, 4096, 2] (fp32).
  d[b,m,n] = || t[b,m] - a[b,n] ||
  forward_loss[b,m]  = min_n d[b,m,n]
  backward_loss[b,n] = min_m d[b,m,n]

Strategy
--------
Shard batch B=16 across 8 cores (2 batches/core). On each core, compute the
squared-distance matrix ONCE (single orientation, targets stationary) with
the PE via the bilinear identity
    d2[m,n] = |t_m|^2 + |a_n|^2 - 2 t_m . a_n
expressed as a K=18 matmul: each fp32 scalar is split into 3 bf16 limbs
(hi/mid/lo) so bf16 matmuls (1 cyc/row on PE) reproduce fp32-level
precision; limb products are ordered large-first so PSUM accumulation
rounds at small magnitude near the minima.

Per [128m x 4096n] block:
  - ScalarE evacuates PSUM to an fp16 SBUF tile (the only other engine that
    can read PSUM, freeing the DVE).
  - forward:  DVE tt-min of the two halves (2x packed) + 1x tensor_reduce.
  - backward: DVE tt-min accumulation into a per-batch [128, 4096] running
    column-min (2x packed).
Backward finalization: negate, GpSimd partition_all_reduce(max) across the
128 partitions, then sqrt(-x) on ScalarE. sqrt only touches final vectors
(sqrt is monotonic, so mins commute with it).
"""

import numpy as np
import ml_dtypes

B, M, N = 16, 4096, 4096
NCORES = 8
BPC = B // NCORES          # batches per core
F = BPC * M                # free width of aug arrays per core
K = 18                     # contraction rows
NB = M // 128              # m-blocks per batch (32)
HALF = 2048                # psum tile free width
BF16 = ml_dtypes.bfloat16

_CACHE = {}


def _build_nc():
    import concourse.mybir as mybir
    import concourse.tile as tile
    from concourse import bacc, bass_isa

    nc = bacc.Bacc(None, target_bir_lowering=False)
    taug_d = nc.declare_dram_parameter("taug", [K, F], mybir.dt.bfloat16, isOutput=False)
    aaug_d = nc.declare_dram_parameter("aaug", [K, F], mybir.dt.bfloat16, isOutput=False)
    fwd_d = nc.declare_dram_parameter("fwd", [BPC, 128, NB], mybir.dt.float32, isOutput=True)
    bwd_d = nc.declare_dram_parameter("bwd", [BPC, N], mybir.dt.float32, isOutput=True)

    f32 = mybir.dt.float32
    f16 = mybir.dt.float16
    fmin = mybir.AluOpType.min
    ax_x = mybir.AxisListType.X
    FCopy = mybir.ActivationFunctionType.Copy
    FSqrt = mybir.ActivationFunctionType.Sqrt

    with tile.TileContext(nc) as tc:
        with (
            tc.tile_pool(name="aug", bufs=1) as augp,
            tc.tile_pool(name="ps", bufs=2, space="PSUM") as psp,
            tc.tile_pool(name="e16", bufs=2) as e16p,
            tc.tile_pool(name="cmb", bufs=2) as cmbp,
            tc.tile_pool(name="accb", bufs=2) as accbp,
            tc.tile_pool(name="pm", bufs=2) as pmp,
            tc.tile_pool(name="fin", bufs=2) as finp,
            tc.tile_pool(name="bfin", bufs=1) as bfinp,
        ):
            ta = augp.tile([K, F], mybir.dt.bfloat16, tag="ta")
            aa = augp.tile([K, F], mybir.dt.bfloat16, tag="aa")
            nc.sync.dma_start(out=ta[:], in_=taug_d[:])
            nc.sync.dma_start(out=aa[:], in_=aaug_d[:])

            for b in range(BPC):
                pm = pmp.tile([128, NB], f32, tag="pm")
                acc = accbp.tile([128, N], f16, tag="acc")
                nc.vector.memset(acc[:], 60000.0)
                for i2 in range(NB // 2):
                    # two m-blocks (i0, i1) share one e16 tile so every DVE op
                    # below covers both via 3D access patterns
                    e16 = e16p.tile([128, 2 * N], f16, tag="e16")
                    for u in range(2):
                        i = 2 * i2 + u
                        lhsT = ta[:, b * M + i * 128 : b * M + (i + 1) * 128]
                        for h in range(2):
                            ps = psp.tile([128, HALF], f32, tag="ps")
                            for j in range(4):
                                nc.tensor.matmul(
                                    ps[:, j * 512 : (j + 1) * 512],
                                    lhsT,
                                    aa[:, b * M + h * HALF + j * 512 : b * M + h * HALF + (j + 1) * 512],
                                    start=True,
                                    stop=True,
                                )
                            nc.scalar.activation(
                                out=e16[:, u * N + h * HALF : u * N + (h + 1) * HALF],
                                in_=ps[:],
                                func=FCopy,
                            )
                    # forward fold tree (2x-packed TT, both blocks per op via
                    # a [128, 2, F] view), then one segmented 1x reduce
                    ev = e16[:].rearrange("p (u n) -> p u n", u=2)
                    c = cmbp.tile([128, 2, HALF], f16, tag="c")
                    nc.vector.tensor_tensor(
                        out=c[:], in0=ev[:, :, 0:HALF], in1=ev[:, :, HALF:N], op=fmin
                    )
                    c2 = cmbp.tile([128, 2, 1024], f16, tag="c2")
                    nc.vector.tensor_tensor(
                        out=c2[:], in0=c[:, :, 0:1024], in1=c[:, :, 1024:HALF], op=fmin
                    )
                    c3 = cmbp.tile([128, 2, 512], f16, tag="c3")
                    nc.vector.tensor_tensor(
                        out=c3[:], in0=c2[:, :, 0:512], in1=c2[:, :, 512:1024], op=fmin
                    )
                    nc.vector.tensor_reduce(
                        out=pm[:, 2 * i2 : 2 * i2 + 2], in_=c3[:], axis=ax_x, op=fmin
                    )
                    # backward: pair the two blocks, then merge into the running
                    # column-min (halves acc traffic)
                    p = cmbp.tile([128, N], f16, tag="p")
                    nc.vector.tensor_tensor(
                        out=p[:], in0=e16[:, 0:N], in1=e16[:, N : 2 * N], op=fmin
                    )
                    nc.vector.tensor_tensor(out=acc[:], in0=acc[:], in1=p[:], op=fmin)

                # forward finalize: clamp + sqrt, out layout [128, NB] (host transposes)
                fc = finp.tile([128, NB], f32, tag="fc")
                nc.vector.tensor_scalar_max(out=fc[:], in0=pm[:], scalar1=0.0)
                fs = finp.tile([128, NB], f32, tag="fs")
                nc.scalar.activation(out=fs[:], in_=fc[:], func=FSqrt)
                nc.sync.dma_start(out=fwd_d[b], in_=fs[:])

                # backward finalize: negate, cross-partition max, sqrt(-x)
                nacc = bfinp.tile([128, N], f16, tag="nacc")
                nc.vector.tensor_scalar_mul(out=nacc[:], in0=acc[:], scalar1=-1.0)
                par = bfinp.tile([128, N], f16, tag="par")
                nc.gpsimd.partition_all_reduce(
                    par[:], nacc[:], channels=128, reduce_op=bass_isa.ReduceOp.max
                )
                row = bfinp.tile([1, N], f16, tag="row")
                nc.vector.tensor_scalar_min(out=row[:], in0=par[0:1, :], scalar1=0.0)
                brow = bfinp.tile([1, N], f32, tag="brow")
                nc.scalar.activation(out=brow[:], in_=row[:], func=FSqrt, scale=-1.0)
                nc.sync.dma_start(out=bwd_d[b], in_=brow[:])

    nc.finalize()
    return nc


def _split3(v):
    """3-way bf16 limb split of fp64 array: h + m + l == v to ~24 mantissa bits."""
    h = v.astype(BF16)
    r = v - h.astype(np.float64)
    m = r.astype(BF16)
    r2 = r - m.astype(np.float64)
    l = r2.astype(BF16)
    return h, m, l


def _make_augs(tp, ap):
    """tp, ap: [nb, M, 2] fp32 -> (taug, aaug) [K, nb*M] bf16."""
    t = tp.astype(np.float64).transpose(2, 0, 1).reshape(2, -1)  # [coord, nb*M]
    a = ap.astype(np.float64).transpose(2, 0, 1).reshape(2, -1)
    n = t.shape[1]

    txh, txm, txl = _split3(t[0])
    tyh, tym, tyl = _split3(t[1])
    t2h, t2m, t2l = _split3(t[0] ** 2 + t[1] ** 2)
    Xh, Xm, Xl = _split3(-2.0 * a[0])
    Yh, Ym, Yl = _split3(-2.0 * a[1])
    a2h, a2m, a2l = _split3(a[0] ** 2 + a[1] ** 2)
    one = np.ones(n, dtype=BF16)

    # Product pairs ordered so the PE's in-instruction fp32 accumulation sees
    # the large terms first (partial sum collapses to ~d2 after k=3, so later
    # roundings happen at small magnitude): t2_h, hh cross terms, a2_h, then
    # the mid/lo correction limbs {hm, mh, hl, lh, mm}.
    taug = np.stack([
        t2h, txh, tyh, one,
        t2m, txh, txm, tyh, tym, one,
        txh, txl, txm, tyh, tyl, tym,
        t2l, one,
    ])
    aaug = np.stack([
        one, Xh, Yh, a2h,
        one, Xm, Xh, Ym, Yh, a2m,
        Xl, Xh, Xm, Yl, Yh, Ym,
        one, a2l,
    ])
    return np.ascontiguousarray(taug), np.ascontiguousarray(aaug)


def run(target_points, actual_points, trace=False, tmpdir=None):
    from concourse.bass_utils import run_bass_kernel_spmd

    tp = np.asarray(target_points, dtype=np.float32)
    ap = np.asarray(actual_points, dtype=np.float32)
    assert tp.shape == (B, M, 2) and ap.shape == (B, N, 2)

    if "nc" not in _CACHE:
        _CACHE["nc"] = _build_nc()
    nc = _CACHE["nc"]

    in_maps = []
    for c in range(NCORES):
        taug, aaug = _make_augs(tp[BPC * c : BPC * (c + 1)], ap[BPC * c : BPC * (c + 1)])
        in_maps.append({"taug": taug, "aaug": aaug})

    res = run_bass_kernel_spmd(
        nc, in_maps, core_ids=list(range(NCORES)), trace=trace, tmpdir=tmpdir
    )

    fwd = np.empty((B, M), dtype=np.float32)
    bwd = np.empty((B, N), dtype=np.float32)
    for c in range(NCORES):
        # fwd device layout [BPC, 128, NB]: element (b, p, i) -> index i*128 + p
        fwd[BPC * c : BPC * (c + 1)] = (
            res.results[c]["fwd"].transpose(0, 2, 1).reshape(BPC, M)
        )
        bwd[BPC * c : BPC * (c + 1)] = res.results[c]["bwd"]
    return (fwd, bwd), res


def kernel(target_points, actual_points):
    (fwd, bwd), _ = run(target_points, actual_points)
    return fwd, bwd


# revision 7
# speedup vs baseline: 1.6285x; 1.0260x over previous
"""Chamfer loss kernel for Trainium2 (8 NeuronCores, batch-parallel).

Problem: target_points [16, 4096, 2], actual_points [16, 4096, 2] (fp32).
  d[b,m,n] = || t[b,m] - a[b,n] ||
  forward_loss[b,m]  = min_n d[b,m,n]
  backward_loss[b,n] = min_m d[b,m,n]

Strategy
--------
Shard batch B=16 across 8 cores (2 batches/core). On each core, compute the
squared-distance matrix ONCE (single orientation, targets stationary) with
the PE via the bilinear identity
    d2[m,n] = |t_m|^2 + |a_n|^2 - 2 t_m . a_n
expressed as a K=18 matmul: each fp32 scalar is split into 3 bf16 limbs
(hi/mid/lo) so bf16 matmuls (1 cyc/row on PE) reproduce fp32-level
precision; limb products are ordered large-first so PSUM accumulation
rounds at small magnitude near the minima.

Per [128m x 4096n] block:
  - ScalarE evacuates PSUM to an fp16 SBUF tile (the only other engine that
    can read PSUM, freeing the DVE).
  - forward:  DVE tt-min of the two halves (2x packed) + 1x tensor_reduce.
  - backward: DVE tt-min accumulation into a per-batch [128, 4096] running
    column-min (2x packed).
Backward finalization: negate, GpSimd partition_all_reduce(max) across the
128 partitions, then sqrt(-x) on ScalarE. sqrt only touches final vectors
(sqrt is monotonic, so mins commute with it).
"""

import numpy as np
import ml_dtypes

B, M, N = 16, 4096, 4096
NCORES = 8
BPC = B // NCORES          # batches per core
F = BPC * M                # free width of aug arrays per core
K = 18                     # contraction rows
NB = M // 128              # m-blocks per batch (32)
HALF = 2048                # psum tile free width
BF16 = ml_dtypes.bfloat16

_CACHE = {}


def _build_nc():
    import concourse.mybir as mybir
    import concourse.tile as tile
    from concourse import bacc, bass_isa

    nc = bacc.Bacc(None, target_bir_lowering=False)
    taug_d = nc.declare_dram_parameter("taug", [K, F], mybir.dt.bfloat16, isOutput=False)
    aaug_d = nc.declare_dram_parameter("aaug", [K, F], mybir.dt.bfloat16, isOutput=False)
    fwd_d = nc.declare_dram_parameter("fwd", [BPC, 128, NB], mybir.dt.float32, isOutput=True)
    bwd_d = nc.declare_dram_parameter("bwd", [BPC, N], mybir.dt.float32, isOutput=True)

    f32 = mybir.dt.float32
    f16 = mybir.dt.float16
    fmin = mybir.AluOpType.min
    fmax = mybir.AluOpType.max
    ax_x = mybir.AxisListType.X
    FCopy = mybir.ActivationFunctionType.Copy
    FSqrt = mybir.ActivationFunctionType.Sqrt

    with tile.TileContext(nc) as tc:
        with (
            tc.tile_pool(name="aug", bufs=1) as augp,
            tc.tile_pool(name="ps", bufs=2, space="PSUM") as psp,
            tc.tile_pool(name="e16", bufs=3) as e16p,
            tc.tile_pool(name="cmb", bufs=2) as cmbp,
            tc.tile_pool(name="accb", bufs=2) as accbp,
            tc.tile_pool(name="pm", bufs=2) as pmp,
            tc.tile_pool(name="fin", bufs=2) as finp,
            tc.tile_pool(name="bfin", bufs=1) as bfinp,
        ):
            ta = augp.tile([K, F], mybir.dt.bfloat16, tag="ta")
            aa = augp.tile([K, F], mybir.dt.bfloat16, tag="aa")
            nc.sync.dma_start(out=ta[:], in_=taug_d[:])
            nc.sync.dma_start(out=aa[:], in_=aaug_d[:])

            for b in range(BPC):
                pm = pmp.tile([128, NB], f32, tag="pm")
                acc = accbp.tile([128, N], f16, tag="acc")
                nc.gpsimd.memset(acc[:], -60000.0)
                for i2 in range(NB // 2):
                    # two m-blocks (i0, i1) share one e16 tile so every DVE op
                    # below covers both via 3D access patterns
                    e16 = e16p.tile([128, 2 * N], f16, tag="e16")
                    for u in range(2):
                        i = 2 * i2 + u
                        lhsT = ta[:, b * M + i * 128 : b * M + (i + 1) * 128]
                        for h in range(2):
                            ps = psp.tile([128, HALF], f32, tag="ps")
                            for j in range(4):
                                nc.tensor.matmul(
                                    ps[:, j * 512 : (j + 1) * 512],
                                    lhsT,
                                    aa[:, b * M + h * HALF + j * 512 : b * M + h * HALF + (j + 1) * 512],
                                    start=True,
                                    stop=True,
                                )
                            nc.scalar.activation(
                                out=e16[:, u * N + h * HALF : u * N + (h + 1) * HALF],
                                in_=ps[:],
                                func=FCopy,
                                scale=-1.0,
                            )
                    # forward fold tree (2x-packed TT, both blocks per op via
                    # a [128, 2, F] view), then one segmented 1x reduce
                    ev = e16[:].rearrange("p (u n) -> p u n", u=2)
                    c = cmbp.tile([128, 2, HALF], f16, tag="c")
                    nc.vector.tensor_tensor(
                        out=c[:], in0=ev[:, :, 0:HALF], in1=ev[:, :, HALF:N], op=fmax
                    )
                    c2 = cmbp.tile([128, 2, 1024], f16, tag="c2")
                    nc.vector.tensor_tensor(
                        out=c2[:], in0=c[:, :, 0:1024], in1=c[:, :, 1024:HALF], op=fmax
                    )
                    c3 = cmbp.tile([128, 2, 512], f16, tag="c3")
                    nc.vector.tensor_tensor(
                        out=c3[:], in0=c2[:, :, 0:512], in1=c2[:, :, 512:1024], op=fmax
                    )
                    nc.vector.tensor_reduce(
                        out=pm[:, 2 * i2 : 2 * i2 + 2], in_=c3[:], axis=ax_x, op=fmax
                    )
                    # backward: pair the two blocks, then merge into the running
                    # column-min (halves acc traffic)
                    p = cmbp.tile([128, N], f16, tag="p")
                    nc.vector.tensor_tensor(
                        out=p[:], in0=e16[:, 0:N], in1=e16[:, N : 2 * N], op=fmax
                    )
                    nc.vector.tensor_tensor(out=acc[:], in0=acc[:], in1=p[:], op=fmax)

                # forward finalize: clamp + sqrt, out layout [128, NB] (host transposes)
                fc = finp.tile([128, NB], f32, tag="fc")
                nc.vector.tensor_scalar_min(out=fc[:], in0=pm[:], scalar1=0.0)
                fs = finp.tile([128, NB], f32, tag="fs")
                nc.scalar.activation(out=fs[:], in_=fc[:], func=FSqrt, scale=-1.0)
                nc.sync.dma_start(out=fwd_d[b], in_=fs[:])

                # backward finalize: negate, cross-partition max, sqrt(-x)
                par = bfinp.tile([128, N], f16, tag="par")
                nc.gpsimd.partition_all_reduce(
                    par[:], acc[:], channels=128, reduce_op=bass_isa.ReduceOp.max
                )
                row = bfinp.tile([1, N], f16, tag="row")
                nc.vector.tensor_scalar_min(out=row[:], in0=par[0:1, :], scalar1=0.0)
                brow = bfinp.tile([1, N], f32, tag="brow")
                nc.scalar.activation(out=brow[:], in_=row[:], func=FSqrt, scale=-1.0)
                nc.sync.dma_start(out=bwd_d[b], in_=brow[:])

    nc.finalize()
    return nc


def _split3(v):
    """3-way bf16 limb split of fp64 array: h + m + l == v to ~24 mantissa bits."""
    h = v.astype(BF16)
    r = v - h.astype(np.float64)
    m = r.astype(BF16)
    r2 = r - m.astype(np.float64)
    l = r2.astype(BF16)
    return h, m, l


def _make_augs(tp, ap):
    """tp, ap: [nb, M, 2] fp32 -> (taug, aaug) [K, nb*M] bf16."""
    t = tp.astype(np.float64).transpose(2, 0, 1).reshape(2, -1)  # [coord, nb*M]
    a = ap.astype(np.float64).transpose(2, 0, 1).reshape(2, -1)
    n = t.shape[1]

    txh, txm, txl = _split3(t[0])
    tyh, tym, tyl = _split3(t[1])
    t2h, t2m, t2l = _split3(t[0] ** 2 + t[1] ** 2)
    Xh, Xm, Xl = _split3(-2.0 * a[0])
    Yh, Ym, Yl = _split3(-2.0 * a[1])
    a2h, a2m, a2l = _split3(a[0] ** 2 + a[1] ** 2)
    one = np.ones(n, dtype=BF16)

    # Product pairs ordered so the PE's in-instruction fp32 accumulation sees
    # the large terms first (partial sum collapses to ~d2 after k=3, so later
    # roundings happen at small magnitude): t2_h, hh cross terms, a2_h, then
    # the mid/lo correction limbs {hm, mh, hl, lh, mm}.
    taug = np.stack([
        t2h, txh, tyh, one,
        t2m, txh, txm, tyh, tym, one,
        txh, txl, txm, tyh, tyl, tym,
        t2l, one,
    ])
    aaug = np.stack([
        one, Xh, Yh, a2h,
        one, Xm, Xh, Ym, Yh, a2m,
        Xl, Xh, Xm, Yl, Yh, Ym,
        one, a2l,
    ])
    return np.ascontiguousarray(taug), np.ascontiguousarray(aaug)


def run(target_points, actual_points, trace=False, tmpdir=None):
    from concourse.bass_utils import run_bass_kernel_spmd

    tp = np.asarray(target_points, dtype=np.float32)
    ap = np.asarray(actual_points, dtype=np.float32)
    assert tp.shape == (B, M, 2) and ap.shape == (B, N, 2)

    if "nc" not in _CACHE:
        _CACHE["nc"] = _build_nc()
    nc = _CACHE["nc"]

    in_maps = []
    for c in range(NCORES):
        taug, aaug = _make_augs(tp[BPC * c : BPC * (c + 1)], ap[BPC * c : BPC * (c + 1)])
        in_maps.append({"taug": taug, "aaug": aaug})

    res = run_bass_kernel_spmd(
        nc, in_maps, core_ids=list(range(NCORES)), trace=trace, tmpdir=tmpdir
    )

    fwd = np.empty((B, M), dtype=np.float32)
    bwd = np.empty((B, N), dtype=np.float32)
    for c in range(NCORES):
        # fwd device layout [BPC, 128, NB]: element (b, p, i) -> index i*128 + p
        fwd[BPC * c : BPC * (c + 1)] = (
            res.results[c]["fwd"].transpose(0, 2, 1).reshape(BPC, M)
        )
        bwd[BPC * c : BPC * (c + 1)] = res.results[c]["bwd"]
    return (fwd, bwd), res


def kernel(target_points, actual_points):
    (fwd, bwd), _ = run(target_points, actual_points)
    return fwd, bwd


# revision 8
# speedup vs baseline: 1.6535x; 1.0154x over previous
"""Chamfer loss kernel for Trainium2 (8 NeuronCores, batch-parallel).

Problem: target_points [16, 4096, 2], actual_points [16, 4096, 2] (fp32).
  d[b,m,n] = || t[b,m] - a[b,n] ||
  forward_loss[b,m]  = min_n d[b,m,n]
  backward_loss[b,n] = min_m d[b,m,n]

Strategy
--------
Shard batch B=16 across 8 cores (2 batches/core). On each core, compute the
squared-distance matrix ONCE (single orientation, targets stationary) with
the PE via the bilinear identity
    d2[m,n] = |t_m|^2 + |a_n|^2 - 2 t_m . a_n
expressed as a K=18 matmul: each fp32 scalar is split into 3 bf16 limbs
(hi/mid/lo) so bf16 matmuls (1 cyc/row on PE) reproduce fp32-level
precision; limb products are ordered large-first so PSUM accumulation
rounds at small magnitude near the minima.

Per [128m x 4096n] block:
  - ScalarE evacuates PSUM to an fp16 SBUF tile (the only other engine that
    can read PSUM, freeing the DVE).
  - forward:  DVE tt-min of the two halves (2x packed) + 1x tensor_reduce.
  - backward: DVE tt-min accumulation into a per-batch [128, 4096] running
    column-min (2x packed).
Backward finalization: negate, GpSimd partition_all_reduce(max) across the
128 partitions, then sqrt(-x) on ScalarE. sqrt only touches final vectors
(sqrt is monotonic, so mins commute with it).
"""

import numpy as np
import ml_dtypes

B, M, N = 16, 4096, 4096
NCORES = 8
BPC = B // NCORES          # batches per core
F = BPC * M                # free width of aug arrays per core
K = 18                     # contraction rows
NB = M // 128              # m-blocks per batch (32)
HALF = 2048                # psum tile free width
BF16 = ml_dtypes.bfloat16

_CACHE = {}


def _build_nc():
    import concourse.mybir as mybir
    import concourse.tile as tile
    from concourse import bacc, bass_isa

    nc = bacc.Bacc(None, target_bir_lowering=False)
    taug_d = nc.declare_dram_parameter("taug", [K, F], mybir.dt.bfloat16, isOutput=False)
    aaug_d = nc.declare_dram_parameter("aaug", [K, F], mybir.dt.bfloat16, isOutput=False)
    fwd_d = nc.declare_dram_parameter("fwd", [BPC, 128, NB], mybir.dt.float32, isOutput=True)
    bwd_d = nc.declare_dram_parameter("bwd", [BPC, N], mybir.dt.float32, isOutput=True)

    f32 = mybir.dt.float32
    f16 = mybir.dt.float16
    fmin = mybir.AluOpType.min
    fmax = mybir.AluOpType.max
    ax_x = mybir.AxisListType.X
    FCopy = mybir.ActivationFunctionType.Copy
    FSqrt = mybir.ActivationFunctionType.Sqrt

    with tile.TileContext(nc) as tc:
        with (
            tc.tile_pool(name="aug", bufs=1) as augp,
            tc.tile_pool(name="ps", bufs=2, space="PSUM") as psp,
            tc.tile_pool(name="e16", bufs=3) as e16p,
            tc.tile_pool(name="cmb", bufs=2) as cmbp,
            tc.tile_pool(name="accb", bufs=2) as accbp,
            tc.tile_pool(name="pm", bufs=2) as pmp,
            tc.tile_pool(name="fin", bufs=2) as finp,
            tc.tile_pool(name="bfin", bufs=1) as bfinp,
        ):
            ta = augp.tile([K, F], mybir.dt.bfloat16, tag="ta")
            aa = augp.tile([K, F], mybir.dt.bfloat16, tag="aa")
            nc.sync.dma_start(out=ta[:], in_=taug_d[:])
            nc.sync.dma_start(out=aa[:], in_=aaug_d[:])

            for b in range(BPC):
                pm = pmp.tile([128, NB], f32, tag="pm")
                acc = accbp.tile([128, N], f16, tag="acc")
                nc.gpsimd.memset(acc[:], -60000.0)
                for i2 in range(NB // 2):
                    # two m-blocks (i0, i1) share one e16 tile so every DVE op
                    # below covers both via 3D access patterns
                    e16 = e16p.tile([128, 2 * N], f16, tag="e16")
                    for u in range(2):
                        i = 2 * i2 + u
                        lhsT = ta[:, b * M + i * 128 : b * M + (i + 1) * 128]
                        for h in range(2):
                            ps = psp.tile([128, HALF], f32, tag="ps")
                            for j in range(4):
                                nc.tensor.matmul(
                                    ps[:, j * 512 : (j + 1) * 512],
                                    lhsT,
                                    aa[:, b * M + h * HALF + j * 512 : b * M + h * HALF + (j + 1) * 512],
                                    start=True,
                                    stop=True,
                                )
                            nc.scalar.activation(
                                out=e16[:, u * N + h * HALF : u * N + (h + 1) * HALF],
                                in_=ps[:],
                                func=FCopy,
                                scale=-1.0,
                            )
                    # forward fold tree (2x-packed TT, both blocks per op via
                    # a [128, 2, F] view), then one segmented 1x reduce
                    ev = e16[:].rearrange("p (u n) -> p u n", u=2)
                    c = cmbp.tile([128, 2, HALF], f16, tag="c")
                    nc.vector.tensor_tensor(
                        out=c[:], in0=ev[:, :, 0:HALF], in1=ev[:, :, HALF:N], op=fmax
                    )
                    c2 = cmbp.tile([128, 2, 1024], f16, tag="c2")
                    nc.vector.tensor_tensor(
                        out=c2[:], in0=c[:, :, 0:1024], in1=c[:, :, 1024:HALF], op=fmax
                    )
                    c3 = cmbp.tile([128, 2, 512], f16, tag="c3")
                    nc.vector.tensor_tensor(
                        out=c3[:], in0=c2[:, :, 0:512], in1=c2[:, :, 512:1024], op=fmax
                    )
                    c4 = cmbp.tile([128, 2, 256], f16, tag="c4")
                    nc.vector.tensor_tensor(
                        out=c4[:], in0=c3[:, :, 0:256], in1=c3[:, :, 256:512], op=fmax
                    )
                    nc.vector.tensor_reduce(
                        out=pm[:, 2 * i2 : 2 * i2 + 2], in_=c4[:], axis=ax_x, op=fmax
                    )
                    # backward: pair the two blocks, then merge into the running
                    # column-min (halves acc traffic)
                    p = cmbp.tile([128, N], f16, tag="p")
                    nc.vector.tensor_tensor(
                        out=p[:], in0=e16[:, 0:N], in1=e16[:, N : 2 * N], op=fmax
                    )
                    nc.vector.tensor_tensor(out=acc[:], in0=acc[:], in1=p[:], op=fmax)

                # forward finalize: clamp + sqrt, out layout [128, NB] (host transposes)
                fc = finp.tile([128, NB], f32, tag="fc")
                nc.vector.tensor_scalar_min(out=fc[:], in0=pm[:], scalar1=0.0)
                fs = finp.tile([128, NB], f32, tag="fs")
                nc.scalar.activation(out=fs[:], in_=fc[:], func=FSqrt, scale=-1.0)
                nc.sync.dma_start(out=fwd_d[b], in_=fs[:])

                # backward finalize: negate, cross-partition max, sqrt(-x)
                par = bfinp.tile([128, N], f16, tag="par")
                nc.gpsimd.partition_all_reduce(
                    par[:], acc[:], channels=128, reduce_op=bass_isa.ReduceOp.max
                )
                row = bfinp.tile([1, N], f16, tag="row")
                nc.vector.tensor_scalar_min(out=row[:], in0=par[0:1, :], scalar1=0.0)
                brow = bfinp.tile([1, N], f32, tag="brow")
                nc.scalar.activation(out=brow[:], in_=row[:], func=FSqrt, scale=-1.0)
                nc.sync.dma_start(out=bwd_d[b], in_=brow[:])

    nc.finalize()
    return nc


def _split3(v):
    """3-way bf16 limb split of fp64 array: h + m + l == v to ~24 mantissa bits."""
    h = v.astype(BF16)
    r = v - h.astype(np.float64)
    m = r.astype(BF16)
    r2 = r - m.astype(np.float64)
    l = r2.astype(BF16)
    return h, m, l


def _make_augs(tp, ap):
    """tp, ap: [nb, M, 2] fp32 -> (taug, aaug) [K, nb*M] bf16."""
    t = tp.astype(np.float64).transpose(2, 0, 1).reshape(2, -1)  # [coord, nb*M]
    a = ap.astype(np.float64).transpose(2, 0, 1).reshape(2, -1)
    n = t.shape[1]

    txh, txm, txl = _split3(t[0])
    tyh, tym, tyl = _split3(t[1])
    t2h, t2m, t2l = _split3(t[0] ** 2 + t[1] ** 2)
    Xh, Xm, Xl = _split3(-2.0 * a[0])
    Yh, Ym, Yl = _split3(-2.0 * a[1])
    a2h, a2m, a2l = _split3(a[0] ** 2 + a[1] ** 2)
    one = np.ones(n, dtype=BF16)

    # Product pairs ordered so the PE's in-instruction fp32 accumulation sees
    # the large terms first (partial sum collapses to ~d2 after k=3, so later
    # roundings happen at small magnitude): t2_h, hh cross terms, a2_h, then
    # the mid/lo correction limbs {hm, mh, hl, lh, mm}.
    taug = np.stack([
        t2h, txh, tyh, one,
        t2m, txh, txm, tyh, tym, one,
        txh, txl, txm, tyh, tyl, tym,
        t2l, one,
    ])
    aaug = np.stack([
        one, Xh, Yh, a2h,
        one, Xm, Xh, Ym, Yh, a2m,
        Xl, Xh, Xm, Yl, Yh, Ym,
        one, a2l,
    ])
    return np.ascontiguousarray(taug), np.ascontiguousarray(aaug)


def run(target_points, actual_points, trace=False, tmpdir=None):
    from concourse.bass_utils import run_bass_kernel_spmd

    tp = np.asarray(target_points, dtype=np.float32)
    ap = np.asarray(actual_points, dtype=np.float32)
    assert tp.shape == (B, M, 2) and ap.shape == (B, N, 2)

    if "nc" not in _CACHE:
        _CACHE["nc"] = _build_nc()
    nc = _CACHE["nc"]

    in_maps = []
    for c in range(NCORES):
        taug, aaug = _make_augs(tp[BPC * c : BPC * (c + 1)], ap[BPC * c : BPC * (c + 1)])
        in_maps.append({"taug": taug, "aaug": aaug})

    res = run_bass_kernel_spmd(
        nc, in_maps, core_ids=list(range(NCORES)), trace=trace, tmpdir=tmpdir
    )

    fwd = np.empty((B, M), dtype=np.float32)
    bwd = np.empty((B, N), dtype=np.float32)
    for c in range(NCORES):
        # fwd device layout [BPC, 128, NB]: element (b, p, i) -> index i*128 + p
        fwd[BPC * c : BPC * (c + 1)] = (
            res.results[c]["fwd"].transpose(0, 2, 1).reshape(BPC, M)
        )
        bwd[BPC * c : BPC * (c + 1)] = res.results[c]["bwd"]
    return (fwd, bwd), res


def kernel(target_points, actual_points):
    (fwd, bwd), _ = run(target_points, actual_points)
    return fwd, bwd


# revision 9
# speedup vs baseline: 1.6708x; 1.0104x over previous
"""Chamfer loss kernel for Trainium2 (8 NeuronCores, batch-parallel).

Problem: target_points [16, 4096, 2], actual_points [16, 4096, 2] (fp32).
  d[b,m,n] = || t[b,m] - a[b,n] ||
  forward_loss[b,m]  = min_n d[b,m,n]
  backward_loss[b,n] = min_m d[b,m,n]

Strategy
--------
Shard batch B=16 across 8 cores (2 batches/core). On each core, compute the
squared-distance matrix ONCE (single orientation, targets stationary) with
the PE via the bilinear identity
    d2[m,n] = |t_m|^2 + |a_n|^2 - 2 t_m . a_n
expressed as a K=18 matmul: each fp32 scalar is split into 3 bf16 limbs
(hi/mid/lo) so bf16 matmuls (1 cyc/row on PE) reproduce fp32-level
precision; limb products are ordered large-first so PSUM accumulation
rounds at small magnitude near the minima.

Per [128m x 4096n] block:
  - ScalarE evacuates PSUM to an fp16 SBUF tile (the only other engine that
    can read PSUM, freeing the DVE).
  - forward:  DVE tt-min of the two halves (2x packed) + 1x tensor_reduce.
  - backward: DVE tt-min accumulation into a per-batch [128, 4096] running
    column-min (2x packed).
Backward finalization: negate, GpSimd partition_all_reduce(max) across the
128 partitions, then sqrt(-x) on ScalarE. sqrt only touches final vectors
(sqrt is monotonic, so mins commute with it).
"""

import numpy as np
import ml_dtypes

B, M, N = 16, 4096, 4096
NCORES = 8
BPC = B // NCORES          # batches per core
F = BPC * M                # free width of aug arrays per core
K = 18                     # contraction rows
NB = M // 128              # m-blocks per batch (32)
HALF = 2048                # psum tile free width
BF16 = ml_dtypes.bfloat16

_CACHE = {}


def _build_nc():
    import concourse.mybir as mybir
    import concourse.tile as tile
    from concourse import bacc, bass_isa

    nc = bacc.Bacc(None, target_bir_lowering=False)
    taug_d = nc.declare_dram_parameter("taug", [K, F], mybir.dt.bfloat16, isOutput=False)
    aaug_d = nc.declare_dram_parameter("aaug", [K, F], mybir.dt.bfloat16, isOutput=False)
    fwd_d = nc.declare_dram_parameter("fwd", [BPC, 128, NB], mybir.dt.float32, isOutput=True)
    bwd_d = nc.declare_dram_parameter("bwd", [BPC, N], mybir.dt.float32, isOutput=True)

    f32 = mybir.dt.float32
    f16 = mybir.dt.float16
    fmin = mybir.AluOpType.min
    fmax = mybir.AluOpType.max
    ax_x = mybir.AxisListType.X
    FCopy = mybir.ActivationFunctionType.Copy
    FSqrt = mybir.ActivationFunctionType.Sqrt

    with tile.TileContext(nc) as tc:
        with (
            tc.tile_pool(name="aug", bufs=1) as augp,
            tc.tile_pool(name="ps", bufs=2, space="PSUM") as psp,
            tc.tile_pool(name="e16", bufs=3) as e16p,
            tc.tile_pool(name="cmb", bufs=2) as cmbp,
            tc.tile_pool(name="accb", bufs=2) as accbp,
            tc.tile_pool(name="pm", bufs=2) as pmp,
            tc.tile_pool(name="fin", bufs=2) as finp,
            tc.tile_pool(name="bfin", bufs=1) as bfinp,
        ):
            ta = augp.tile([K, F], mybir.dt.bfloat16, tag="ta")
            aa = augp.tile([K, F], mybir.dt.bfloat16, tag="aa")
            for hb in range(4):
                sl = slice(hb * (F // 4), (hb + 1) * (F // 4))
                nc.sync.dma_start(out=aa[:, sl], in_=aaug_d[:, sl])
                nc.sync.dma_start(out=ta[:, sl], in_=taug_d[:, sl])

            for b in range(BPC):
                pm = pmp.tile([128, NB], f32, tag="pm")
                acc = accbp.tile([128, N], f16, tag="acc")
                nc.gpsimd.memset(acc[:], -60000.0)
                for i2 in range(NB // 2):
                    # two m-blocks (i0, i1) share one e16 tile so every DVE op
                    # below covers both via 3D access patterns
                    e16 = e16p.tile([128, 2 * N], f16, tag="e16")
                    for u in range(2):
                        i = 2 * i2 + u
                        lhsT = ta[:, b * M + i * 128 : b * M + (i + 1) * 128]
                        for h in range(2):
                            ps = psp.tile([128, HALF], f32, tag="ps")
                            for j in range(4):
                                nc.tensor.matmul(
                                    ps[:, j * 512 : (j + 1) * 512],
                                    lhsT,
                                    aa[:, b * M + h * HALF + j * 512 : b * M + h * HALF + (j + 1) * 512],
                                    start=True,
                                    stop=True,
                                )
                            nc.scalar.activation(
                                out=e16[:, u * N + h * HALF : u * N + (h + 1) * HALF],
                                in_=ps[:],
                                func=FCopy,
                                scale=-1.0,
                            )
                    # forward fold tree (2x-packed TT, both blocks per op via
                    # a [128, 2, F] view), then one segmented 1x reduce
                    ev = e16[:].rearrange("p (u n) -> p u n", u=2)
                    c = cmbp.tile([128, 2, HALF], f16, tag="c")
                    nc.vector.tensor_tensor(
                        out=c[:], in0=ev[:, :, 0:HALF], in1=ev[:, :, HALF:N], op=fmax
                    )
                    c2 = cmbp.tile([128, 2, 1024], f16, tag="c2")
                    nc.vector.tensor_tensor(
                        out=c2[:], in0=c[:, :, 0:1024], in1=c[:, :, 1024:HALF], op=fmax
                    )
                    c3 = cmbp.tile([128, 2, 512], f16, tag="c3")
                    nc.vector.tensor_tensor(
                        out=c3[:], in0=c2[:, :, 0:512], in1=c2[:, :, 512:1024], op=fmax
                    )
                    c4 = cmbp.tile([128, 2, 256], f16, tag="c4")
                    nc.vector.tensor_tensor(
                        out=c4[:], in0=c3[:, :, 0:256], in1=c3[:, :, 256:512], op=fmax
                    )
                    nc.vector.tensor_reduce(
                        out=pm[:, 2 * i2 : 2 * i2 + 2], in_=c4[:], axis=ax_x, op=fmax
                    )
                    # backward: pair the two blocks, then merge into the running
                    # column-min (halves acc traffic)
                    p = cmbp.tile([128, N], f16, tag="p")
                    nc.vector.tensor_tensor(
                        out=p[:], in0=e16[:, 0:N], in1=e16[:, N : 2 * N], op=fmax
                    )
                    nc.vector.tensor_tensor(out=acc[:], in0=acc[:], in1=p[:], op=fmax)

                # forward finalize: clamp + sqrt, out layout [128, NB] (host transposes)
                fc = finp.tile([128, NB], f32, tag="fc")
                nc.vector.tensor_scalar_min(out=fc[:], in0=pm[:], scalar1=0.0)
                fs = finp.tile([128, NB], f32, tag="fs")
                nc.scalar.activation(out=fs[:], in_=fc[:], func=FSqrt, scale=-1.0)
                nc.sync.dma_start(out=fwd_d[b], in_=fs[:])

                # backward finalize: negate, cross-partition max, sqrt(-x)
                par = bfinp.tile([128, N], f16, tag="par")
                nc.gpsimd.partition_all_reduce(
                    par[:], acc[:], channels=128, reduce_op=bass_isa.ReduceOp.max
                )
                row = bfinp.tile([1, N], f16, tag="row")
                nc.vector.tensor_scalar_min(out=row[:], in0=par[0:1, :], scalar1=0.0)
                brow = bfinp.tile([1, N], f32, tag="brow")
                nc.scalar.activation(out=brow[:], in_=row[:], func=FSqrt, scale=-1.0)
                nc.sync.dma_start(out=bwd_d[b], in_=brow[:])

    nc.finalize()
    return nc


def _split3(v):
    """3-way bf16 limb split of fp64 array: h + m + l == v to ~24 mantissa bits."""
    h = v.astype(BF16)
    r = v - h.astype(np.float64)
    m = r.astype(BF16)
    r2 = r - m.astype(np.float64)
    l = r2.astype(BF16)
    return h, m, l


def _make_augs(tp, ap):
    """tp, ap: [nb, M, 2] fp32 -> (taug, aaug) [K, nb*M] bf16."""
    t = tp.astype(np.float64).transpose(2, 0, 1).reshape(2, -1)  # [coord, nb*M]
    a = ap.astype(np.float64).transpose(2, 0, 1).reshape(2, -1)
    n = t.shape[1]

    txh, txm, txl = _split3(t[0])
    tyh, tym, tyl = _split3(t[1])
    t2h, t2m, t2l = _split3(t[0] ** 2 + t[1] ** 2)
    Xh, Xm, Xl = _split3(-2.0 * a[0])
    Yh, Ym, Yl = _split3(-2.0 * a[1])
    a2h, a2m, a2l = _split3(a[0] ** 2 + a[1] ** 2)
    one = np.ones(n, dtype=BF16)

    # Product pairs ordered so the PE's in-instruction fp32 accumulation sees
    # the large terms first (partial sum collapses to ~d2 after k=3, so later
    # roundings happen at small magnitude): t2_h, hh cross terms, a2_h, then
    # the mid/lo correction limbs {hm, mh, hl, lh, mm}.
    taug = np.stack([
        t2h, txh, tyh, one,
        t2m, txh, txm, tyh, tym, one,
        txh, txl, txm, tyh, tyl, tym,
        t2l, one,
    ])
    aaug = np.stack([
        one, Xh, Yh, a2h,
        one, Xm, Xh, Ym, Yh, a2m,
        Xl, Xh, Xm, Yl, Yh, Ym,
        one, a2l,
    ])
    return np.ascontiguousarray(taug), np.ascontiguousarray(aaug)


def run(target_points, actual_points, trace=False, tmpdir=None):
    from concourse.bass_utils import run_bass_kernel_spmd

    tp = np.asarray(target_points, dtype=np.float32)
    ap = np.asarray(actual_points, dtype=np.float32)
    assert tp.shape == (B, M, 2) and ap.shape == (B, N, 2)

    if "nc" not in _CACHE:
        _CACHE["nc"] = _build_nc()
    nc = _CACHE["nc"]

    in_maps = []
    for c in range(NCORES):
        taug, aaug = _make_augs(tp[BPC * c : BPC * (c + 1)], ap[BPC * c : BPC * (c + 1)])
        in_maps.append({"taug": taug, "aaug": aaug})

    res = run_bass_kernel_spmd(
        nc, in_maps, core_ids=list(range(NCORES)), trace=trace, tmpdir=tmpdir
    )

    fwd = np.empty((B, M), dtype=np.float32)
    bwd = np.empty((B, N), dtype=np.float32)
    for c in range(NCORES):
        # fwd device layout [BPC, 128, NB]: element (b, p, i) -> index i*128 + p
        fwd[BPC * c : BPC * (c + 1)] = (
            res.results[c]["fwd"].transpose(0, 2, 1).reshape(BPC, M)
        )
        bwd[BPC * c : BPC * (c + 1)] = res.results[c]["bwd"]
    return (fwd, bwd), res


def kernel(target_points, actual_points):
    (fwd, bwd), _ = run(target_points, actual_points)
    return fwd, bwd
